# revision 2
# baseline (speedup 1.0000x reference)
"""Cross multi-head attention Trainium2 Bass kernel v2 (head-sharded).

Problem: nn_CrossMutiHeadAttention (B=4, SQ=SKV=2048, d_model=1024, H=8,
d_k=64, d_v=128), fp32 in/out.

Sharding (8 cores, no collectives): core c handles batch b=c//2 and head
GROUP g=c%2 (heads 4g..4g+3) for ALL 2048 query rows. Each core computes
K^T/V projections only for its 4 heads, attention for its 4 heads, and a
PARTIAL output projection against its 512 rows of Wo. The host sums the
two partial outputs per batch at gather time.

v2 changes vs v1:
- enc/pre/wq/wk/wv are passed from the host as bf16; enc^T / pre^T are
  produced by DMA XBAR transposes (dma_start_transpose) straight from
  DRAM instead of PE transposes + DVE casts/copies. Saves ~14 us PE and
  ~36 us DVE per core; the DMA engines were idle.
- Weights DMA directly into their SBUF tiles (no f32 staging + DVE cast).
  Wo is declared f32r and DMA'd as raw f32.
- The two heads of a pair have their score matmuls (K=64) issued
  back-to-back at PE row tiles (0,0)/(64,0), so they run concurrently in
  the PE array (the array is otherwise half idle for K=64).
- exp tiles hold (head-even, head-odd) in the two j-halves; one
  activation covers both heads and the softmax denominator needs just
  ONE ones-matmul per head (the j-half is a full kv partial sum).
"""

from contextlib import ExitStack

import numpy as np

import concourse.bass as bass
import concourse.mybir as mybir
from concourse import bacc
from concourse.bass_utils import run_bass_kernel_spmd
from concourse.tile import TileContext

F32 = mybir.dt.float32
F32R = mybir.dt.float32r
BF16 = mybir.dt.bfloat16
FP16 = mybir.dt.float16

P = 128
B, SQ, SKV, DM = 4, 2048, 2048, 1024
H, DK, DV = 8, 64, 128
HG = H // 2            # 4 heads per core (head group)
HPG = HG // 2          # 2 head pairs per core
NQK = HG * DK          # 256 = per-core Wq/Wk column count
NV = HG * DV           # 512 = per-core Wv column count
WOR = NV               # 512 = per-core Wo row count
CO = DM // P           # 8 contraction chunks for projections
WCO = WOR // P         # 4 contraction chunks for Wo
NKV = SKV // P         # 16 kv chunks of 128
N_CORES = 8

EXP_SCALE = 1.0 / np.sqrt(DK).astype(np.float32)  # 0.125
import os
PAIRED = os.environ.get("KV2_PAIRED", "1") == "1"
DRIP_Q = os.environ.get("KV2_DRIP_Q", "0") == "1"
DRIP_Y = os.environ.get("KV2_DRIP_Y", "1") == "1"
NO_PV = os.environ.get("KV2_NO_PV", "0") == "1"
NO_NORM = os.environ.get("KV2_NO_NORM", "0") == "1"
NO_OUT = os.environ.get("KV2_NO_OUT", "0") == "1"


def build(loop_phase="all"):
    nc = bacc.Bacc()
    enc = nc.declare_dram_parameter("enc", [SKV, DM], BF16, isOutput=False)
    pre = nc.declare_dram_parameter("pre", [SQ, DM], BF16, isOutput=False)
    wq = nc.declare_dram_parameter("wq", [DM, NQK], BF16, isOutput=False)
    wk = nc.declare_dram_parameter("wk", [DM, NQK], BF16, isOutput=False)
    wv = nc.declare_dram_parameter("wv", [DM, NV], BF16, isOutput=False)
    wo = nc.declare_dram_parameter("wo", [WOR, DM], F32, isOutput=False)
    n_it = nc.declare_dram_parameter("n_it", [1, 1], mybir.dt.uint32, isOutput=False)
    out = nc.declare_dram_parameter("out", [SQ, DM], FP16, isOutput=True)

    with ExitStack() as ctx:
        tc = ctx.enter_context(TileContext(nc))
        ec = ctx.enter_context
        if True:
            cpool = ec(tc.tile_pool(name="const", bufs=1))
            wk_pool = ec(tc.tile_pool(name="wk", bufs=1))
            wq_pool = ec(tc.tile_pool(name="wq", bufs=1))
            wv_pool = ec(tc.tile_pool(name="wvp", bufs=1))
            wo_pool = ec(tc.tile_pool(name="wop", bufs=2))
            ktst_pool = ec(tc.tile_pool(name="ktst", bufs=1))
            qt_pool = ec(tc.tile_pool(name="qt", bufs=1))
            tblk_pool = ec(tc.tile_pool(name="tblk", bufs=4))
            v_pool = ec(tc.tile_pool(name="vpool", bufs=1))
            exp_pool = ec(tc.tile_pool(name="exp", bufs=10))
            acc_pool = ec(tc.tile_pool(name="acc", bufs=2))
            ot_pool = ec(tc.tile_pool(name="ot", bufs=2))
            y_pool = ec(tc.tile_pool(name="ysb", bufs=2))
            r_pool = ec(tc.tile_pool(name="rsm", bufs=4))
            ps_st = ec(tc.tile_pool(name="ps_st", bufs=2, space="PSUM"))
            ps_mm = ec(tc.tile_pool(name="ps_mm", bufs=2, space="PSUM"))
            ps_aux = ec(tc.tile_pool(name="ps_aux", bufs=2, space="PSUM"))
            ones = cpool.tile([P, 1], FP16, tag="ones")
            nc.gpsimd.memset(ones[:], 1.0)
            nit_sb = cpool.tile([1, 1], mybir.dt.uint32, tag="nit")
            nc.sync.dma_start(nit_sb[:], n_it[:])

            regs = []
            for eng_t in mybir.ALL_ENGINES:
                r = nc.alloc_register(eng_t, f"nit_{eng_t.name}")
                nc.engines[eng_t].reg_load(r, nit_sb[0:1, 0:1])
                regs.append(r)
            n_val = bass.RegisterHandles(regs)

            state = {}

            def ph1():
                # K^T resident: [128 (d of 2 heads), pair, SKV] bf16
                kt_sb = ktst_pool.tile([P, HPG, SKV], BF16, tag="ktst")
                state["kt_sb"] = kt_sb
                v_sb = v_pool.tile([P, NKV, NV], FP16, tag="v")
                state["v_sb"] = v_sb
                wk_sb = wk_pool.tile([P, CO, NQK], BF16, tag="wk")
                state["wk_sb"] = wk_sb
                wv_sb = wv_pool.tile([P, CO, NV], BF16, tag="wvp")
                state["wv_sb"] = wv_sb

                def load_w_kv():
                    # weights DMA straight into bf16 tiles (row co*128+p of
                    # the DRAM weight lands at [p, co, :])
                    nc.sync.dma_start(
                        wk_sb[:], wk.rearrange("(co p) n -> p co n", p=P)
                    )
                    nc.sync.dma_start(
                        wv_sb[:], wv.rearrange("(co p) n -> p co n", p=P)
                    )

                # ---- ph1: per 512-row enc block: DMA transpose, K/V proj;
                # pre-block transposes and Q-projections are interleaved so
                # no XBAR transpose remains inside the attention stretch ----
                pts = {}
                for blk in range(SKV // 512):
                    et = tblk_pool.tile([P, CO, 512], BF16, tag="tblk")
                    nc.sync.dma_start_transpose(
                        et[:], enc[blk * 512 : (blk + 1) * 512, :]
                    )
                    if blk == 0:
                        load_w_kv()
                    if not DRIP_Q:
                        # queue this q-block's transpose right behind et's
                        pt = tblk_pool.tile([P, CO, 512], BF16, tag="tblk")
                        nc.sync.dma_start_transpose(
                            pt[:], pre[blk * 512 : (blk + 1) * 512, :]
                        )
                        pts[blk] = pt
                    # K^T proj: psum [128 pair-d, 512 kv] -> kt_sb
                    for hp in range(HPG):
                        kp = ps_mm.tile([P, 512], F32, tag="mm512")
                        for co in range(CO):
                            nc.tensor.matmul(
                                kp[:],
                                lhsT=wk_sb[:, co, hp * P : (hp + 1) * P],
                                rhs=et[:, co, :],
                                start=(co == 0),
                                stop=(co == CO - 1),
                            )
                        nc.scalar.copy(
                            kt_sb[:, hp, blk * 512 : (blk + 1) * 512], kp[:]
                        )
                    # V proj: psum [128 kv, 512 dv-cols (4 heads)]
                    for t in range(4):
                        vp = ps_mm.tile([P, 512], F32, tag="mm512")
                        for co in range(CO):
                            nc.tensor.matmul(
                                vp[:],
                                lhsT=et[:, co, t * P : (t + 1) * P],
                                rhs=wv_sb[:, co, :],
                                start=(co == 0),
                                stop=(co == CO - 1),
                            )
                        nc.scalar.copy(v_sb[:, blk * 4 + t, :], vp[:])
                    if not DRIP_Q and blk > 0:
                        # previous block's Q-proj (its pt landed during this
                        # block's K/V matmuls)
                        ptp = pts.pop(blk - 1)
                        for hp in range(HPG):
                            ph23_proj(blk - 1, ptp, hp)
                if not DRIP_Q:
                    ptp = pts.pop(3)
                    for hp in range(HPG):
                        ph23_proj(3, ptp, hp)

            def load_wq():
                wq_sb = wq_pool.tile([P, CO, NQK], BF16, tag="wq")
                state["wq_sb"] = wq_sb
                nc.sync.dma_start(wq_sb[:], wq.rearrange("(co p) n -> p co n", p=P))
                qt_sb = qt_pool.tile([P, HPG, SQ], BF16, tag="qt")
                state["qt_sb"] = qt_sb

            def ph23_transpose(qc):
                # DMA-transpose one 512-row pre block
                pt = tblk_pool.tile([P, CO, 512], BF16, tag="tblk")
                nc.sync.dma_start_transpose(pt[:], pre[qc * 512 : (qc + 1) * 512, :])
                return pt

            def ph23_proj(qc, pt, hp):
                # project Q^T for one head pair of block qc
                wq_sb, qt_sb = state["wq_sb"], state["qt_sb"]
                qp = ps_aux.tile([P, 512], F32, tag="aux")
                for co in range(CO):
                    nc.tensor.matmul(
                        qp[:],
                        lhsT=wq_sb[:, co, hp * P : (hp + 1) * P],
                        rhs=pt[:, co, :],
                        start=(co == 0),
                        stop=(co == CO - 1),
                    )
                nc.vector.tensor_copy(
                    qt_sb[:, hp, qc * 512 : (qc + 1) * 512], qp[:]
                )

            def phwo():
                # Wo DMA'd as plain f32 (an F32R-typed DMA corrupts SBUF on
                # HW, and walrus requires f32r matmul inputs to be produced
                # by an f32r-rounding instruction) then DVE-rounded to f32r.
                wo_stg = wo_pool.tile([P, WCO, DM], F32, tag="wostg")
                wo_r = wo_pool.tile([P, WCO, DM], F32R, tag="wop")
                state["wo_r"] = wo_r
                nc.sync.dma_start(wo_stg[:], wo.rearrange("(co p) n -> p co n", p=P))
                nc.vector.tensor_copy(wo_r[:], wo_stg[:])

            def ph45_unpaired():
                # ---- v1-style attention: one head at a time, st j-halves
                # are kv-parity chunks; no row-tile pairing ----
                kt_sb, v_sb, qt_sb = state["kt_sb"], state["v_sb"], state["qt_sb"]
                wo_r = state["wo_r"]
                SKEW = 4
                pending = []
                yp_q = []
                qp_q = []

                def consume(otp, acc, ot_qc, h, kvh, ex):
                    for j in range(2):
                        kvc = 2 * kvh + j
                        nc.tensor.matmul(
                            otp[:],
                            lhsT=v_sb[:, kvc, h * DV : (h + 1) * DV],
                            rhs=ex[:, j, :],
                            start=(kvc == 0),
                            stop=(kvc == NKV - 1),
                        )
                    if kvh == 7:
                        sm = ps_aux.tile([1, 512], F32, tag="aux")
                        for j in range(2):
                            nc.tensor.matmul(
                                sm[:], lhsT=ones[:], rhs=acc[:, j, :],
                                start=(j == 0), stop=(j == 1),
                            )
                        rr = r_pool.tile([1, 512], F32, tag="r")
                        nc.vector.reciprocal(rr[:], sm[:])
                        rb = r_pool.tile([P, 512], F32, tag="rb")
                        nc.gpsimd.partition_broadcast(rb[:], rr[:])
                        nc.vector.tensor_mul(ot_qc[:, h, :], otp[:], rb[:])

                def proj_out_group(qc, ot_qc, qt, n2):
                    row0 = qc * 512 + qt * P
                    nsl = slice(n2 * 512, (n2 + 1) * 512)
                    yp = ps_aux.tile([P, 512], F32, tag="aux")
                    for hc in range(WCO):
                        nc.tensor.matmul(
                            yp[:],
                            lhsT=ot_qc[:, hc, qt * P : (qt + 1) * P],
                            rhs=wo_r[:, hc, nsl],
                            start=(hc == 0),
                            stop=(hc == WCO - 1),
                        )
                    ty = y_pool.tile([P, 512], FP16, tag="y")
                    nc.vector.tensor_copy(ty[:], yp[:])
                    nc.sync.dma_start(out[row0 : row0 + P, nsl], ty[:])

                for qc in range(SQ // 512):
                    ot_qc = ot_pool.tile([P, HG, 512], F32R, tag="potr")
                    if qc + 1 < SQ // 512:
                        pt_next = ph23_transpose(qc + 1)
                        qp_q.extend(
                            [lambda qcn=qc + 1, hp=hp, pt=pt_next: ph23_proj(
                                qcn, pt, hp)
                             for hp in range(HPG)]
                        )
                    if state.get("prev") is not None:
                        pqc, pot = state["prev"]
                        yp_q.extend(
                            [lambda qt=qt, n2=n2, pqc=pqc, pot=pot:
                             proj_out_group(pqc, pot, qt, n2)
                             for qt in range(4) for n2 in range(2)]
                        )
                    state["prev"] = (qc, ot_qc)
                    for h in range(HG):
                        hp, odd = h // 2, h % 2
                        base = 64 * odd
                        ktp = kt_sb[:, hp, :]
                        otp = ps_mm.tile([P, 512], F32, tag="mm512")
                        acc = acc_pool.tile([P, 2, 512], FP16, tag="acc")
                        for kvh in range(8):
                            st = ps_st.tile([P, 2, 512], F32, tag="st")
                            qsl = slice(qc * 512, (qc + 1) * 512)
                            for j in range(2):
                                kvc = 2 * kvh + j
                                nc.tensor.matmul(
                                    st[:, j, :],
                                    lhsT=ktp[base : base + 64,
                                             kvc * P : (kvc + 1) * P],
                                    rhs=qt_sb[base : base + 64, hp, qsl],
                                    start=True,
                                    stop=True,
                                )
                            ex = exp_pool.tile([P, 2, 512], FP16, tag="exp")
                            nc.scalar.activation(
                                ex[:], st[:],
                                mybir.ActivationFunctionType.Exp,
                                bias=0.0, scale=float(EXP_SCALE),
                            )
                            if kvh == 0:
                                nc.vector.tensor_copy(acc[:], ex[:])
                            else:
                                nc.vector.tensor_add(acc[:], acc[:], ex[:])
                            pending.append((otp, acc, ot_qc, h, kvh, ex))
                            if len(pending) > SKEW:
                                consume(*pending.pop(0))
                            if (
                                kvh % 2 == 1
                                and (h >= 1 or kvh >= 5)
                                and yp_q
                            ):
                                yp_q.pop(0)()
                            elif kvh % 4 == 2 and qp_q:
                                qp_q.pop(0)()
                for item in pending:
                    consume(*item)
                pending.clear()
                for f in qp_q:
                    f()
                qp_q.clear()
                for f in yp_q:
                    f()
                yp_q.clear()
                pqc, pot = state["prev"]
                state["prev"] = None
                for qt in range(4):
                    for n2 in range(2):
                        proj_out_group(pqc, pot, qt, n2)

            def ph45():
                # ---- attention (paired heads) + interleaved out-projection
                kt_sb, v_sb, qt_sb = state["kt_sb"], state["v_sb"], state["qt_sb"]
                wo_r = state["wo_r"]
                SKEW = 6
                pending = []  # per (qc, hp, kvc) entries across pairs
                yp_q = []     # deferred out-projection groups (one per yp tile)
                qp_q = []     # deferred Q-proj emissions for the next qc

                def consume(otp0, otp1, acc, ot_qc, hp, kvc, ex):
                    if NO_PV:
                        return
                    h0, h1 = 2 * hp, 2 * hp + 1
                    nc.tensor.matmul(
                        otp0[:],
                        lhsT=v_sb[:, kvc, h0 * DV : (h0 + 1) * DV],
                        rhs=ex[:, 0, :],
                        start=(kvc == 0),
                        stop=(kvc == NKV - 1),
                    )
                    nc.tensor.matmul(
                        otp1[:],
                        lhsT=v_sb[:, kvc, h1 * DV : (h1 + 1) * DV],
                        rhs=ex[:, 1, :],
                        start=(kvc == 0),
                        stop=(kvc == NKV - 1),
                    )
                    if NO_NORM:
                        return
                    if kvc == NKV - 1:
                        # softmax denominators: each j-half of acc is the
                        # full kv partial sum for one head. (All reads for
                        # Pool/DVE stay at base partition 0 — partition-
                        # offset views crash the GPSIMD/DVE ucode on HW.)
                        for j, otp in enumerate((otp0, otp1)):
                            sm = ps_aux.tile([1, 512], F32, tag="aux")
                            nc.tensor.matmul(
                                sm[:], lhsT=ones[:], rhs=acc[:, j, :],
                                start=True, stop=True,
                            )
                            rr = r_pool.tile([1, 512], F32, tag="r")
                            nc.vector.reciprocal(rr[:], sm[:])
                            rb = r_pool.tile([P, 512], F32, tag="rb")
                            nc.gpsimd.partition_broadcast(rb[:], rr[:])
                            nc.vector.tensor_mul(
                                ot_qc[:, 2 * hp + j, :], otp[:], rb[:]
                            )

                def proj_out_group(qc, ot_qc, qt, n2):
                    if NO_OUT:
                        return
                    # one [128q, 512n] tile of Y_partial[qc] = ot_qc.T @ Wo_g
                    row0 = qc * 512 + qt * P
                    nsl = slice(n2 * 512, (n2 + 1) * 512)
                    yp = ps_aux.tile([P, 512], F32, tag="aux")
                    for hc in range(WCO):
                        nc.tensor.matmul(
                            yp[:],
                            lhsT=ot_qc[:, hc, qt * P : (qt + 1) * P],
                            rhs=wo_r[:, hc, nsl],
                            start=(hc == 0),
                            stop=(hc == WCO - 1),
                        )
                    ty = y_pool.tile([P, 512], FP16, tag="y")
                    nc.vector.tensor_copy(ty[:], yp[:])
                    nc.sync.dma_start(out[row0 : row0 + P, nsl], ty[:])

                for qc in range(SQ // 512):
                    ot_qc = ot_pool.tile([P, HG, 512], F32R, tag="potr")
                    if DRIP_Q and qc + 1 < SQ // 512:
                        # DMA-transpose the next q block now; its Q-proj
                        # matmuls are drip-fed into this chunk's kv loops
                        pt_next = ph23_transpose(qc + 1)
                        qp_q.extend(
                            [lambda qcn=qc + 1, hp=hp, pt=pt_next: ph23_proj(
                                qcn, pt, hp)
                             for hp in range(HPG)]
                        )
                    if DRIP_Y and state.get("prev") is not None:
                        # queue the previous chunk's out-projection; it is
                        # drip-fed into this chunk's ACT-paced kv loops
                        pqc, pot = state["prev"]
                        yp_q.extend(
                            [lambda qt=qt, n2=n2, pqc=pqc, pot=pot:
                             proj_out_group(pqc, pot, qt, n2)
                             for qt in range(4) for n2 in range(2)]
                        )
                    elif not DRIP_Y and state.get("prev") is not None:
                        # burst the previous chunk's out-projection here
                        pqc, pot = state["prev"]
                        for qt in range(4):
                            for n2 in range(2):
                                proj_out_group(pqc, pot, qt, n2)
                    state["prev"] = (qc, ot_qc)
                    for hp in range(HPG):
                        ktp = kt_sb[:, hp, :]
                        otp0 = ps_mm.tile([P, 512], F32, tag="mm512")
                        otp1 = ps_mm.tile([P, 512], F32, tag="mm512")
                        acc = acc_pool.tile([P, 2, 512], FP16, tag="acc")
                        for kvc in range(NKV):
                            st = ps_st.tile([P, 2, 512], F32, tag="st")
                            qsl = slice(qc * 512, (qc + 1) * 512)
                            # paired K=64 score matmuls: row tiles (0,0) and
                            # (64,0) -> concurrent in the PE array
                            nc.tensor.matmul(
                                st[:, 0, :],
                                lhsT=ktp[0:64, kvc * P : (kvc + 1) * P],
                                rhs=qt_sb[0:64, hp, qsl],
                                start=True,
                                stop=True,
                            )
                            nc.tensor.matmul(
                                st[:, 1, :],
                                lhsT=ktp[64:128, kvc * P : (kvc + 1) * P],
                                rhs=qt_sb[64:128, hp, qsl],
                                start=True,
                                stop=True,
                            )
                            ex = exp_pool.tile([P, 2, 512], FP16, tag="exp")
                            nc.scalar.activation(
                                ex[:],
                                st[:],
                                mybir.ActivationFunctionType.Exp,
                                bias=0.0,
                                scale=float(EXP_SCALE),
                            )
                            if kvc == 0:
                                nc.vector.tensor_copy(acc[:], ex[:])
                            else:
                                nc.vector.tensor_add(acc[:], acc[:], ex[:])
                            pending.append((otp0, otp1, acc, ot_qc, hp, kvc, ex))
                            if len(pending) > SKEW:
                                consume(*pending.pop(0))
                            # drip-feed deferred PE work into the ACT-paced
                            # kv loop: out-proj of the previous chunk (only
                            # once its last heads' normalize has landed),
                            # and Q-proj of the next chunk
                            if (
                                kvc % 2 == 1
                                and (hp == 1 or kvc >= 9)
                                and yp_q
                            ):
                                yp_q.pop(0)()
                            elif kvc % 8 == 4 and qp_q:
                                qp_q.pop(0)()
                for item in pending:
                    consume(*item)
                pending.clear()
                for f in qp_q:
                    f()
                qp_q.clear()
                for f in yp_q:
                    f()
                yp_q.clear()
                pqc, pot = state["prev"]
                state["prev"] = None
                for qt in range(4):
                    for n2 in range(2):
                        proj_out_group(pqc, pot, qt, n2)

            def prep():
                load_wq()
                if DRIP_Q:
                    pt0 = ph23_transpose(0)
                    for hp in range(HPG):
                        ph23_proj(0, pt0, hp)

            def attn():
                if PAIRED:
                    ph45()
                else:
                    ph45_unpaired()

            def body():
                prep()
                phwo()
                ph1()
                attn()

            if loop_phase == "none":
                body()
            elif loop_phase == "all":
                with tc.For_i(
                    0, n_val, 1, hint_engines=tuple(mybir.ALL_ENGINES)
                ) as _i:
                    body()
            elif loop_phase == "ph1":
                # loop K^T/V projection only (transposes + matmuls + copies)
                with tc.For_i(
                    0, n_val, 1, hint_engines=tuple(mybir.ALL_ENGINES)
                ) as _i:
                    ph1()
                prep()
                phwo()
                attn()
            elif loop_phase == "ph45":
                # everything resident once; loop the attention+out section
                prep()
                phwo()
                ph1()
                with tc.For_i(
                    0, n_val, 1, hint_engines=tuple(mybir.ALL_ENGINES)
                ) as _i:
                    attn()
            elif loop_phase == "noattn":
                prep()
                ph1()
                phwo()
            elif loop_phase == "empty":
                ph1()
                prep()
                phwo()
                with tc.For_i(
                    0, n_val, 1, hint_engines=tuple(mybir.ALL_ENGINES)
                ) as _i:
                    nc.gpsimd.memset(ones[:], 1.0)
                attn()
            elif loop_phase == "prep":
                # loop Q-proj/transpose front-end only
                ph1()
                phwo()
                with tc.For_i(
                    0, n_val, 1, hint_engines=tuple(mybir.ALL_ENGINES)
                ) as _i:
                    prep()
                attn()
            else:
                raise ValueError(loop_phase)
    nc.finalize()
    return nc


_NC_CACHE = None


def _get_nc():
    global _NC_CACHE
    if _NC_CACHE is None:
        _NC_CACHE = build()
    return _NC_CACHE


def run_sharded(inputs: dict, n_iters: int = 1):
    """Shard full inputs over 8 cores, run, gather full output.

    Core c -> (batch c//2, head group c%2). Each core returns a partial
    [SQ, DM] output (its 4 heads through its 512 rows of Wo); the two
    partials per batch sum to the full output (host-side all-reduce).

    Returns (full_output [B,SQ,DM] f32, raw BassKernelResults).
    """
    import ml_dtypes

    bf16 = ml_dtypes.bfloat16
    enc_full = np.asarray(inputs["encoder_output"], dtype=np.float32)
    pre_full = np.asarray(inputs["pre_output"], dtype=np.float32)
    wq = np.asarray(inputs["Wq"], dtype=np.float32)
    wk = np.asarray(inputs["Wk"], dtype=np.float32)
    wv = np.asarray(inputs["Wv"], dtype=np.float32)
    wo = np.ascontiguousarray(np.asarray(inputs["Wo"], dtype=np.float32))
    enc_bf = enc_full.astype(bf16)
    pre_bf = pre_full.astype(bf16)
    wq_bf = wq.astype(bf16)
    wk_bf = wk.astype(bf16)
    wv_bf = wv.astype(bf16)
    nit = np.array([[n_iters]], dtype=np.uint32)

    in_maps = []
    for c in range(N_CORES):
        b, g = c // 2, c % 2
        in_maps.append(
            {
                "enc": np.ascontiguousarray(enc_bf[b]),
                "pre": np.ascontiguousarray(pre_bf[b]),
                "wq": np.ascontiguousarray(wq_bf[:, g * NQK : (g + 1) * NQK]),
                "wk": np.ascontiguousarray(wk_bf[:, g * NQK : (g + 1) * NQK]),
                "wv": np.ascontiguousarray(wv_bf[:, g * NV : (g + 1) * NV]),
                "wo": np.ascontiguousarray(wo[g * WOR : (g + 1) * WOR, :]),
                "n_it": nit,
            }
        )
    res = run_bass_kernel_spmd(_get_nc(), in_maps, list(range(N_CORES)))
    full = np.empty((B, SQ, DM), dtype=np.float32)
    for b in range(B):
        full[b] = res.results[2 * b]["out"]
        full[b] += res.results[2 * b + 1]["out"]
    return full, res


def kernel(**inputs) -> np.ndarray:
    full, _ = run_sharded(inputs, n_iters=1)
    return full
